# revision 2
# baseline (speedup 1.0000x reference)
"""Bass/Tile TRN2 kernel v3 for nn_BasicRNN: out = scan(tanh(x@Wx + h@Wh) + h) @ Wout.

Data-parallel over batch across 8 NeuronCores (32 rows/core); recurrence
sequential in time per core.

Architecture (v3): ONE persistent PSUM bank holds P_t * 2^11 directly.
Every contribution enters via PE matmuls into the same 128 columns (m, b):
  - xproj as DIFFERENCED x: dx_t = x_t - x_{t-1} (dx_0 = x_0), so
    P gains xp_t - xp_{t-1} each step via 24 N=32 matmuls (fp16 hi/lo
    3-term, weights at 2^11 scale); the bias feeds in once at t=0.
  - recurrence: u_hi, u_lo (fp16) and u8 (e4m3) matmuls against
    Wh_hi*2^11 (fp16) and Wh_lo8 = e4m3((Wh*2^11 - Wh_hi)*2).
tanh reads the PSUM bank directly on ScalarE with scale=2^-11 (its free
affine), producing uf; DVE derives u_hi = fp16(uf), u_lo = uf - u_hi,
u8 = e4m3(uf/2), h += uf and the fp16 hist slice.  outproj is group-batched
single-pass fp16.  Numerics emulated in numpy: rel_fro ~2.3e-3.

The design minimizes the serial dependency cycle (measured ~270ns per
cross-engine handoff): cycle = tanh -> u-casts -> matmul block -> tanh,
~1.3-1.6us/step; everything else (outproj, h, hist) hangs off-cycle.
"""

import sys

sys.path.insert(0, "/opt/trn_rl_repo")

from collections import deque

import numpy as np

import concourse.bass as bass  # noqa: F401
import concourse.tile as tile
from concourse import bacc, mybir
from concourse.bass_utils import run_bass_kernel_spmd

FP = mybir.dt.float32
F16 = mybir.dt.float16
F8 = mybir.dt.float8e4
TANH = mybir.ActivationFunctionType.Tanh
ALU = mybir.AluOpType

B, D, T, H, OUT = 256, 256, 256, 512, 256
NCORES = 8
BC = B // NCORES   # 32 batch rows per core
P = 128
DC = D // P        # 2 d-chunks
HC = H // P        # 4 h-chunks
SC = 2048.0        # 2^11 recurrence-path scale
ISC = 1.0 / SC


def build(T_=T, G=8, reps=1, fill_per_step=2, banks=1,
          parts=("dx", "hi", "lo", "lo8", "pointwise", "hh", "outproj")):
    parts = set(parts)
    NG = T_ // G
    GB = G * BC            # 256
    TPM = P // BC          # 4 timesteps per outproj M-chunk
    MCG = GB // P          # 2 outproj M-chunks per group
    HB = HC * BC           # 128
    TQ = T_ // 4 if T_ % 4 == 0 else T_
    NQ = T_ // TQ
    assert T_ % G == 0 and GB <= 512

    nc = bacc.Bacc("TRN2", target_bir_lowering=False, debug=False, num_devices=NCORES)

    x_d = nc.declare_dram_parameter("x", [BC, D, T_], FP, isOutput=False)
    wx_d = nc.declare_dram_parameter("Wx", [D, H], FP, isOutput=False)
    wh_d = nc.declare_dram_parameter("Wh", [H, H], FP, isOutput=False)
    b_d = nc.declare_dram_parameter("b", [H], FP, isOutput=False)
    wo_d = nc.declare_dram_parameter("Wout", [H, OUT], FP, isOutput=False)
    bo_d = nc.declare_dram_parameter("bout", [OUT], FP, isOutput=False)
    init_d = nc.declare_dram_parameter("init_state", [1, H], FP, isOutput=False)
    out_d = nc.declare_dram_parameter("out", [BC, T_, OUT], FP, isOutput=True)

    with tile.TileContext(nc) as tc:
        with (
            tc.tile_pool(name="const", bufs=1) as const,
            tc.tile_pool(name="dxbuf", bufs=1) as dxbuf,
            tc.tile_pool(name="xq32", bufs=2) as xq32_pool,
            tc.tile_pool(name="dxf32", bufs=2) as dxf_pool,
            tc.tile_pool(name="h0p", bufs=1) as h0p,
            tc.tile_pool(name="hist", bufs=3) as hist_pool,
            tc.tile_pool(name="uhip", bufs=3) as uhi_pool,
            tc.tile_pool(name="ulop", bufs=3) as ulo_pool,
            tc.tile_pool(name="u8p", bufs=3) as u8_pool,
            tc.tile_pool(name="ufp", bufs=3) as uf_pool,
            tc.tile_pool(name="hfp", bufs=3) as hf_pool,
            tc.tile_pool(name="stg", bufs=4) as stg_pool,
            tc.tile_pool(name="maccp", bufs=1, space="PSUM") as macc_pool,
            tc.tile_pool(name="opp", bufs=2, space="PSUM") as op_psum,
        ):
            # ---------------- prologue: weights ----------------
            wh_hi, wh_lo8 = [], []
            for k in range(HC):
                f = const.tile([P, H], FP, name=f"whf{k}")
                nc.sync.dma_start(out=f[:, :], in_=wh_d[k * P:(k + 1) * P, :])
                hi = const.tile([P, H], F16, name=f"whh{k}")
                nc.vector.tensor_scalar_mul(hi[:, :], f[:, :], SC)
                r = const.tile([P, H], FP, name=f"whr{k}")
                nc.vector.scalar_tensor_tensor(
                    r[:, :], f[:, :], SC, hi[:, :], ALU.mult, ALU.subtract
                )
                lo8 = const.tile([P, H], F8, name=f"whl{k}")
                nc.vector.tensor_scalar_mul(lo8[:, :], r[:, :], 2.0)
                wh_hi.append(hi)
                wh_lo8.append(lo8)

            wx_hi, wx_lo = [], []
            for d in range(DC):
                f = const.tile([P, H], FP, name=f"wxf{d}")
                nc.sync.dma_start(out=f[:, :], in_=wx_d[d * P:(d + 1) * P, :])
                hi = const.tile([P, H], F16, name=f"wxh{d}")
                nc.vector.tensor_scalar_mul(hi[:, :], f[:, :], SC)
                lo = const.tile([P, H], F16, name=f"wxl{d}")
                nc.vector.scalar_tensor_tensor(
                    lo[:, :], f[:, :], SC, hi[:, :], ALU.mult, ALU.subtract
                )
                wx_hi.append(hi)
                wx_lo.append(lo)

            wo16 = []
            for k in range(HC):
                f = const.tile([P, OUT], FP, name=f"wof{k}")
                nc.sync.dma_start(out=f[:, :], in_=wo_d[k * P:(k + 1) * P, :])
                w16 = const.tile([P, OUT], F16, name=f"wo{k}")
                nc.vector.tensor_copy(w16[:, :], f[:, :])
                wo16.append(w16)

            bf = const.tile([1, H], FP, name="bf")
            nc.sync.dma_start(out=bf[:, :], in_=b_d[:].rearrange("(o h) -> o h", o=1))
            b_hi = const.tile([1, H], F16, name="bhi")
            nc.vector.tensor_scalar_mul(b_hi[:, :], bf[:, :], SC)
            b_lo = const.tile([1, H], F16, name="blo")
            nc.vector.scalar_tensor_tensor(
                b_lo[:, :], bf[:, :], SC, b_hi[:, :], ALU.mult, ALU.subtract
            )
            bof = const.tile([1, OUT], FP, name="bof")
            nc.sync.dma_start(out=bof[:, :], in_=bo_d[:].rearrange("(o h) -> o h", o=1))
            bo_hi = const.tile([1, OUT], F16, name="bohi")
            nc.vector.tensor_copy(bo_hi[:, :], bof[:, :])
            bo_lo = const.tile([1, OUT], F16, name="bolo")
            nc.vector.tensor_sub(bo_lo[:, :], bof[:, :], bo_hi[:, :])

            ones = const.tile([1, 512], F16, name="ones")
            nc.vector.memset(ones[:, :], 1.0)
            zrow = const.tile([1, P], F16, name="zrow")
            nc.vector.memset(zrow[:, :], 0.0)

            init_sb = const.tile([P, HC], FP, name="initsb")
            nc.sync.dma_start(
                out=init_sb[:, :], in_=init_d[0, :].rearrange("(c p) -> p c", p=P)
            )

            # ACT table warmup (tanh set) off the timed path
            warm = const.tile([1, P], FP, name="warm")
            nc.vector.memset(warm[:, :], 0.0)
            nc.scalar.activation(warm[:, :], warm[:, :], TANH)

            # -------- prologue: x -> differenced dx fp16 hi/lo, (t, b) -----
            dxh = [dxbuf.tile([P, BC * T_], F16, name=f"dxh{d}") for d in range(DC)]
            dxl = [dxbuf.tile([P, BC * T_], F16, name=f"dxl{d}") for d in range(DC)]
            xq_prev = [None, None]
            for q in range(NQ):
                for d in range(DC):
                    xq = xq32_pool.tile([P, BC * TQ], FP, name=f"xq{q}_{d}", tag=f"xq{d}")
                    src = x_d[:, d * P:(d + 1) * P, q * TQ:(q + 1) * TQ].rearrange(
                        "b d t -> d b t"
                    )
                    nc.sync.dma_start(out=xq.rearrange("p (b t) -> p b t", b=BC), in_=src)
                    xqv = xq.rearrange("p (b t) -> p t b", b=BC)
                    dxf = dxf_pool.tile([P, BC * TQ], FP, name=f"dxf{q}_{d}", tag=f"dxf{d}")
                    dxfv = dxf.rearrange("p (t b) -> p t b", b=BC)
                    nc.vector.tensor_sub(dxfv[:, 1:TQ, :], xqv[:, 1:TQ, :], xqv[:, 0:TQ - 1, :])
                    if q == 0:
                        nc.vector.tensor_copy(dxfv[:, 0, :], xqv[:, 0, :])
                    else:
                        pv = xq_prev[d].rearrange("p (b t) -> p t b", b=BC)
                        nc.vector.tensor_sub(dxfv[:, 0, :], xqv[:, 0, :], pv[:, TQ - 1, :])
                    dh = dxh[d][:, q * TQ * BC:(q + 1) * TQ * BC]
                    dl = dxl[d][:, q * TQ * BC:(q + 1) * TQ * BC]
                    nc.vector.tensor_copy(dh, dxf[:, :])
                    nc.vector.tensor_sub(dl, dxf[:, :], dh)
                    xq_prev[d] = xq

            # ---------------- per-run body ----------------
            def body():
                h0_f = h0p.tile([P, HB], FP, name="h0f")
                nc.vector.memset(h0_f[:, :], 0.0)
                for c in range(HC):
                    nc.vector.tensor_scalar_add(
                        h0_f[:, c * BC:(c + 1) * BC],
                        h0_f[:, c * BC:(c + 1) * BC],
                        init_sb[:, c:c + 1],
                    )
                uhi0 = h0p.tile([P, HB], F16, name="uhi0")
                nc.vector.tensor_copy(uhi0[:, :], h0_f[:, :])
                ulo0 = h0p.tile([P, HB], F16, name="ulo0")
                nc.vector.tensor_sub(ulo0[:, :], h0_f[:, :], uhi0[:, :])
                u80 = h0p.tile([P, HB], F8, name="u80")
                nc.vector.tensor_scalar_mul(u80[:, :], h0_f[:, :], 0.5)

                # persistent P*2^11 accumulator(s).  banks=1: one PSUM bank
                # holding all (m, b) cols.  banks=2: m01 / m23 in separate
                # banks so each half's tanh fires as soon as its half of the
                # matmul pack lands (shorter dependency cycle).
                maccs = [macc_pool.tile([P, 512], FP, name=f"macc{i}")
                         for i in range(banks)]
                for mt in maccs:
                    nc.tensor.matmul(
                        out=mt[:, 0:(HB // banks)], lhsT=zrow[0:1, :],
                        rhs=ones[0:1, 0:(HB // banks)],
                        start=True, stop=False, skip_group_check=True,
                    )

                def macc_ap(m):
                    # columns for m-chunk m within its bank
                    if banks == 1:
                        return maccs[0][:, m * BC:(m + 1) * BC]
                    return maccs[m // 2][:, (m % 2) * BC:(m % 2 + 1) * BC]

                hist_tiles = {}
                fillers = deque()

                def outproj_thunks(g):
                    hist = hist_tiles[g]
                    ths = []
                    for mc in range(MCG):
                        ops = op_psum.tile([P, 512], FP, name=f"op{g}_{mc}", tag="op")

                        def mm_pair(ks, fst, mc=mc, g=g, hist=hist, ops=ops):
                            for k in ks:
                                lhsT = hist[:, k * GB + mc * P: k * GB + (mc + 1) * P]
                                nc.tensor.matmul(
                                    out=ops[:, 0:OUT], lhsT=lhsT, rhs=wo16[k][:, :],
                                    start=fst, stop=False,
                                )
                                fst = False

                        def mm_bias(mc=mc, g=g, ops=ops):
                            for brow in (bo_hi, bo_lo):
                                nc.tensor.matmul(
                                    out=ops[:, 0:OUT], lhsT=ones[0:1, 0:P],
                                    rhs=brow[0:1, :], start=False, stop=False,
                                )

                        def tail(mc=mc, g=g, ops=ops):
                            stg = stg_pool.tile([P, OUT], FP, name=f"st{g}_{mc}", tag="stg")
                            nc.scalar.copy(stg[:, :], ops[:, 0:OUT])
                            t0 = g * G + mc * TPM
                            dst = out_d[:, t0:t0 + TPM, :].rearrange("b t o -> t b o")
                            nc.sync.dma_start(out=dst, in_=stg[:, :])

                        ths.append(lambda f=mm_pair: f((0, 1), True))
                        ths.append(lambda f=mm_pair: f((2, 3), False))
                        ths.append(mm_bias)
                        ths.append(tail)
                    return ths

                prev_f = h0_f[:, :]
                prev_uhi, prev_ulo, prev_u8 = uhi0, ulo0, u80

                for t in range(T_):
                    g, tl = divmod(t, G)
                    if tl == 0:
                        while fillers:
                            fillers.popleft()()
                        hist_tiles[g] = hist_pool.tile(
                            [P, G * HB], F16, name=f"hist{g}", tag="hist"
                        )
                        if g >= 1 and "outproj" in parts:
                            fillers.extend(outproj_thunks(g - 1))

                    # ---- PE pack: all matmuls accumulate into macc ----
                    # dx first (gated only on the previous tanh's PSUM read)
                    if "dx" in parts:
                        for m in range(HC):
                            out_ap = macc_ap(m)
                            for d in range(DC):
                                for lhsT, rhs in (
                                    (wx_hi[d], dxh[d]),
                                    (wx_hi[d], dxl[d]),
                                    (wx_lo[d], dxh[d]),
                                ):
                                    nc.tensor.matmul(
                                        out=out_ap,
                                        lhsT=lhsT[:, m * P:(m + 1) * P],
                                        rhs=rhs[:, t * BC:(t + 1) * BC],
                                        start=False, stop=False, skip_group_check=True,
                                    )
                        if t == 0:
                            for m in range(HC):
                                for brow in (b_hi, b_lo):
                                    nc.tensor.matmul(
                                        out=macc_ap(m),
                                        lhsT=brow[0:1, m * P:(m + 1) * P],
                                        rhs=ones[0:1, 0:BC],
                                        start=False, stop=False, skip_group_check=True,
                                    )
                    # k-major; within k, low-k (early-available u chunks) first
                    for k in range(HC):
                        for kind, rhs_t in (("hi", prev_uhi), ("lo", prev_ulo),
                                            ("lo8", prev_u8)):
                            if kind not in parts:
                                continue
                            wsel = wh_lo8[k] if kind == "lo8" else wh_hi[k]
                            for m in range(HC):
                                nc.tensor.matmul(
                                    out=macc_ap(m),
                                    lhsT=wsel[:, m * P:(m + 1) * P],
                                    rhs=rhs_t[:, k * BC:(k + 1) * BC],
                                    start=False, stop=False, skip_group_check=True,
                                )

                    if "pointwise" not in parts:
                        for _ in range(fill_per_step):
                            if fillers:
                                fillers.popleft()()
                        continue

                    # ---- tanh reads the PSUM bank(s) directly ----
                    uf = uf_pool.tile([P, HB], FP, name=f"uf{t}", tag="uf")
                    uhi = uhi_pool.tile([P, HB], F16, name=f"uhi{t}", tag="uhi")
                    ulo = ulo_pool.tile([P, HB], F16, name=f"ulo{t}", tag="ulo")
                    u8 = u8_pool.tile([P, HB], F8, name=f"u8{t}", tag="u8")
                    HBB = HB // banks
                    for bk in range(banks):
                        sl = slice(bk * HBB, (bk + 1) * HBB)
                        nc.scalar.activation(uf[:, sl], maccs[bk][:, 0:HBB],
                                             TANH, scale=ISC)
                        nc.vector.tensor_copy(uhi[:, sl], uf[:, sl])
                        nc.vector.tensor_sub(ulo[:, sl], uf[:, sl], uhi[:, sl])
                        nc.vector.tensor_scalar_mul(u8[:, sl], uf[:, sl], 0.5)

                    if "hh" in parts:
                        hf = hf_pool.tile([P, HB], FP, name=f"hf{t}", tag="hf")
                        nc.vector.tensor_add(hf[:, :], uf[:, :], prev_f)
                        hdst = hist_tiles[g].rearrange(
                            "p (c t b) -> p c t b", c=HC, t=G
                        )[:, :, tl, :]
                        nc.vector.tensor_copy(hdst, hf.rearrange("p (c b) -> p c b", c=HC))
                        prev_f = hf[:, :]

                    prev_uhi, prev_ulo, prev_u8 = uhi, ulo, u8

                    for _ in range(fill_per_step):
                        if fillers:
                            fillers.popleft()()

                while fillers:
                    fillers.popleft()()
                if "outproj" in parts:
                    for th in outproj_thunks(NG - 1):
                        th()

            if reps > 4:
                with tc.For_i(0, reps, 1):
                    body()
            else:
                for _ in range(reps):
                    body()

    nc.compile()
    return nc


_NC_CACHE = {}


def _get_nc(T_=T, G=8, reps=1, parts=None, banks=None):
    if parts is None:
        parts = ("dx", "hi", "lo", "lo8", "pointwise", "hh", "outproj")
    if banks is None:
        banks = 1
    key = (T_, G, reps, tuple(parts), banks)
    if key not in _NC_CACHE:
        _NC_CACHE[key] = build(T_, G, reps, parts=parts, banks=banks)
    return _NC_CACHE[key]


def run(inputs, T_=T, G=8, reps=1):
    nc = _get_nc(T_, G, reps)
    x = np.ascontiguousarray(np.asarray(inputs["x"], dtype=np.float32))
    shared = {
        k: np.ascontiguousarray(np.asarray(inputs[k], dtype=np.float32))
        for k in ("Wx", "Wh", "b", "Wout", "bout", "init_state")
    }
    core_ids = list(range(NCORES))
    in_maps = [{"x": x[c * BC:(c + 1) * BC], **shared} for c in core_ids]
    res = run_bass_kernel_spmd(nc, in_maps, core_ids)
    out = np.concatenate([res.results[c]["out"] for c in core_ids], axis=0)
    return out


def kernel(**inputs):
    return run(inputs)


if __name__ == "__main__":
    import time

    t0 = time.time()
    _get_nc()
    print(f"build: {time.time() - t0:.1f}s")
